# revision 41
# baseline (speedup 1.0000x reference)
"""GAT layer kernel for Trainium2, 8 NeuronCores.

Problem: nn_GATLayer (B=4, N=2048, IN_F=256, OUT_F=64, H=8).

Key algebra: softmax over j of (src[b,i,h] + dst[b,j,h]) masked by adj[b,i,j].
src[b,i,h] is constant over j, so it cancels in the softmax:
    out[b,i,(h,f)] = (adj[b,i,:] @ g[b,:, (h,f)]) / (adj[b,i,:] @ e[b,:,h])
with hfeat = x@W (per-head features), dst[j,h] = x[j,:] @ (W . attn_dst)[:,h],
e = exp(dst), g = e * hfeat.  attn_src is mathematically irrelevant.

Sharding: 8 cores = 4 batches x 2 row-halves of i (softmax is over j only,
so row-sharding of i needs no communication).

Numerics/perf design (final):
  - fp16 projection (x, W, wdst in fp16; psum f32).  wdst = W . attn_dst
    folded on host and packed into the same dram tensor as W (one DMA).
  - e = exp(dst + ln(3/16)) in fp16; the global 3/16 scale cancels in the
    softmax ratio and keeps g inside e4m3 range.
  - g = e*hfeat written DIRECTLY as fp8 e4m3 by the multiply; the numerator
    is a SINGLE fp8 DoubleRow pass (no hi/lo split, no cast/subtract
    pipeline).  Measured absmax-rel err ~1.5e-2 < 2e-2 gate.
  - Wave-0 denominators are plain f16 matmuls inside the aggregation loop;
    wave-1 denominators are DoubleRow matmuls on an fp8 copy of e and are
    hoisted (with one batched reciprocal) off the finalize tail chains.
  - dst matmuls are issued DSTLEAD j-chunks ahead of the hfeat matmuls so
    the exp's PSUM read never blocks the next chunk's PSUM write (the dep
    tracker is tile-granular).
  - Elementwise per j-chunk: exp batched per pair (ACT); the e*hfeat
    multiply runs on DVE straight from PSUM for most chunks, and via
    ACT-evict(psum->fp16 sbuf) + Pool multiply for POOLJC chunks.
    Finalize (num * 1/den) likewise DVE or ACT+Pool per POOLFIN.
  - Two waves of i-chunks: 2 interleaved with the projection (aggregation
    LAG j-chunks behind), 6 after, completions staggered so fins/stores
    drain progressively; the last i-chunk is column-split into two psum
    tiles so its first fin overlaps its second half's matmuls.
  - 8 input DMAs total (w, 3 xt slabs, 4 adjT groups) keep the shared
    HWDGE/DMA engines ahead of consumption; PE pre-warm dummy matmuls pin
    pe_busy_start early so the p-state ramp completes behind the input
    DMA latency.
"""

import numpy as np
import ml_dtypes

B, N, IN_F, OUT_F, H = 4, 2048, 256, 64, 8
HF = H * OUT_F            # 512 concat features
NCORES = 8
ROWS = B * N // NCORES    # 1024 destination rows per core
P = 128
IC = ROWS // P            # 8 i-chunks per core
JC = N // P               # 16 j-chunks
KC = IN_F // P            # 2 k-chunks
JG = 8                    # DoubleRow j-chunk pairs
AG = 4                    # adjT DMA groups (4 j-chunks each)
WAVE = 2                  # i-chunks in the first (interleaved) wave
NDUMMY = 2                # PE p-state pre-warm matmuls (sets pe_busy_start)
LAG = 8                   # j-chunks the aggregation trails projection by
ESCALE = 0.1875           # global scale on e; cancels in the softmax ratio
WD = HF + H               # packed W columns (hfeat + dst)

POOLJC = (2, 4, 6, 9, 11)       # j-chunks whose multiply goes ACT-evict+Pool
DSTLEAD = 2               # j-chunks the dst matmuls lead the hfeat matmuls by
BDENS = 4                 # wave-1 den pairs issued before wave0 pair 7
BIC3 = False              # ic3's pairs 0-6 run at the wave boundary
POOLFIN = (0, 2)                # i-chunks whose finalize goes ACT-evict+Pool
# xt DMA slabs (j-chunk ranges) interleaved with adjT groups:
#   w, s0, s1, g0, s2, g1, g2, g3
XSLABS = ((0, 2), (2, 5), (5, 8), (8, 16))
FIN7SPLIT = True          # column-split the last ic across two psum tiles

F16 = np.float16
FP8 = ml_dtypes.float8_e4m3

_CACHE = {}


def _bcast_last(ap, n):
    """View ap with an extra innermost broadcast (stride-0) dim of size n."""
    ap2 = ap.unsqueeze(len(ap.shape))
    return ap2.broadcast_to(tuple(ap.shape) + (n,))


def _build():
    import concourse.mybir as mybir
    import concourse.tile as tile
    from concourse import bacc

    f32 = mybir.dt.float32
    f16 = mybir.dt.float16
    fp8 = mybir.dt.float8e4
    MULT = mybir.AluOpType.mult
    DR = mybir.MatmulPerfMode.DoubleRow
    COPY = mybir.ActivationFunctionType.Copy
    EXP = mybir.ActivationFunctionType.Exp

    nc = bacc.Bacc(trn_type="TRN2", debug=False, target_bir_lowering=False)

    adjt_d = nc.dram_tensor("adjt", [N, ROWS], fp8, kind="ExternalInput")
    w_d = nc.dram_tensor("w", [P, KC * WD], f16, kind="ExternalInput")
    xt_d = nc.dram_tensor("xt", [P, JC * KC * P], f16, kind="ExternalInput")
    out_d = nc.dram_tensor("out", [P, IC * HF], f16, kind="ExternalOutput")

    with tile.TileContext(nc) as tc:
        with (
            tc.tile_pool(name="setup", bufs=1) as setup,
            tc.tile_pool(name="gpool", bufs=1) as gpool,
            tc.tile_pool(name="scratch", bufs=2) as scr,
            tc.tile_pool(name="evict", bufs=3) as evp,
            tc.tile_pool(name="adjT", bufs=1) as adjTp,
            tc.tile_pool(name="warm", bufs=1) as warmp,
            tc.tile_pool(name="ps_num", bufs=WAVE, space="PSUM") as psnum,
            tc.tile_pool(name="ps_h", bufs=5, space="PSUM") as psh,
            tc.tile_pool(name="ps_dd", bufs=1, space="PSUM") as psdd,
            tc.tile_pool(name="nsbp", bufs=2) as nsbp,
        ):
            # --- PE pre-warm: dummies into pF0; its first real matmul
            # re-starts the accumulation group, so no extra bank is used ---
            pFs = [psnum.tile([P, HF], f32, tag="num", name=f"pF0_{k}")
                   for k in range(WAVE)]
            junk = warmp.tile([P, 1], f16)
            nc.vector.memset(junk[:], 0.0)
            ebias = warmp.tile([P, 1], f32)
            nc.vector.memset(ebias[:], float(np.log(ESCALE)))
            junk_l = junk[:].broadcast_to((P, P))
            junk_rhs = junk[:].unsqueeze(1).broadcast_to((P, 4, P))
            for _ in range(NDUMMY):
                nc.tensor.matmul(pFs[0][:], junk_l, junk_rhs,
                                 start=True, stop=True, skip_group_check=True)

            # --- input streams via HWDGE (sync), EDF-ordered ---
            w_sb = setup.tile([P, KC, WD], f16)
            w_kc = [w_sb[:, kc] for kc in range(KC)]
            xT_sb = setup.tile([P, JC, KC, P], f16)
            adjT_g = []
            for G in range(AG):
                adjT_g.append(adjTp.tile([P, 4, ROWS], fp8, tag=f"adjt{G}",
                                         name=f"adjt{G}"))

            w_v = w_d.rearrange("p (kc n) -> p kc n", kc=KC)
            xt_v = xt_d.rearrange("p (jc kc j) -> p jc kc j", jc=JC, kc=KC)

            def load_slab(s):
                j0, j1 = XSLABS[s]
                nc.sync.dma_start(
                    xT_sb[:, j0:j1], xt_v[:, j0:j1])

            def load_adjt(G):
                nc.sync.dma_start(
                    adjT_g[G][:],
                    adjt_d[G * 4 * P:(G + 1) * 4 * P, :].rearrange(
                        "(jc jp) i -> jp jc i", jp=P),
                )

            def adjT_pair(c):
                # [P, 2, ROWS] view of j-chunk pair c
                t = 2 * (c % 2)
                return adjT_g[c // 2][:, t:t + 2]

            nc.sync.dma_start(w_sb[:], w_v[:])
            load_slab(0)
            load_slab(1)
            load_slab(2)
            load_adjt(0)
            load_slab(3)
            for G in range(1, AG):
                load_adjt(G)

            def xT(kc, jc):
                return xT_sb[:, jc, kc, :]

            g_hi = gpool.tile([P, JC, HF], fp8)
            e_sb = gpool.tile([P, JC, H], f16)
            e8_sb = gpool.tile([P, JC, H], fp8)
            # dst logits AND denominators share ONE psum bank: dst's single
            # start=True marks the whole bank pending-zero, so den matmuls
            # accumulate with start=False onto zeroed bytes
            pdd = psdd.tile([P, JC + IC, H], f32)
            pden = pdd[:, JC:JC + IC, :]

            def pdst(jc):
                return pdd[:, jc, :]

            phs = {}

            def dst_mm(jc):
                for kc in range(KC):
                    nc.tensor.matmul(
                        pdst(jc), xT(kc, jc), w_kc[kc][:, HF:WD],
                        start=(jc == 0 and kc == 0), stop=(kc == KC - 1),
                        skip_group_check=True,
                    )

            def proj(jc):
                if DSTLEAD == 0:
                    dst_mm(jc)
                ph = psh.tile([P, HF], f32, tag="hfeat")
                phs[jc] = ph
                for kc in range(KC):
                    nc.tensor.matmul(
                        ph[:], xT(kc, jc), w_kc[kc][:, 0:HF],
                        start=(kc == 0), stop=(kc == KC - 1),
                    )

            def exp_pair(c):
                # exp for j-chunks 2c, 2c+1 in one ACT instruction
                nc.scalar.activation(
                    e_sb[:, 2 * c:2 * c + 2, :], pdd[:, 2 * c:2 * c + 2, :],
                    EXP, bias=ebias[:],
                )

            def e8_copy(c):
                # batched fp8 half-copies of e for the wave-1 DoubleRow
                # denominator matmuls (Pool; off the DVE critical path)
                h = slice(0, 8) if c == 3 else slice(8, JC)
                nc.gpsimd.tensor_copy(e8_sb[:, h], e_sb[:, h])

            hfs = {}

            def mult_evict(jc):
                # eager psum->sbuf eviction for Pool-path chunks: frees the
                # ph bank as soon as possible (ACT, ahead of later exps)
                ph = phs.pop(jc)
                hf_sb = evp.tile([P, HF], f16, tag="evict")
                nc.scalar.activation(hf_sb[:], ph[:], COPY)
                hfs[jc] = hf_sb

            def mult(jc):
                # g_hi[jc] = e[jc] (bcast over F) * hfeat[jc], fp8 out
                e3 = _bcast_last(e_sb[:, jc, :], OUT_F)
                g3 = g_hi[:, jc, :].rearrange("p (h f) -> p h f", h=H)
                if jc in POOLJC:
                    hf_sb = hfs.pop(jc)
                    nc.gpsimd.tensor_tensor(
                        g3, hf_sb[:].rearrange("p (h f) -> p h f", h=H), e3,
                        op=MULT)
                else:
                    h3 = phs.pop(jc)[:].rearrange("p (h f) -> p h f", h=H)
                    nc.vector.tensor_tensor(g3, h3, e3, op=MULT)

            def mm_group(pF, ic, c):
                """Single-pass DoubleRow numerator and f16 denominator for
                j-chunk pair c (j-chunks 2c, 2c+1) into i-chunk ic."""
                ap = adjT_pair(c)
                nc.tensor.matmul(
                    pF[:], ap[:, :, ic * P:(ic + 1) * P],
                    g_hi[:, 2 * c:2 * c + 2, :],
                    start=(c == 0), stop=(c == JG - 1),
                    perf_mode=DR, skip_group_check=True,
                )
                for t in range(2):
                    jc = 2 * c + t
                    nc.tensor.matmul(
                        pden[:, ic, :], ap[:, t, ic * P:(ic + 1) * P],
                        e_sb[:, jc, :],
                        start=False, stop=(jc == JC - 1),
                        skip_group_check=True,
                    )

            def fin_mul(ic, pF, rc_ap, nsb, cols=None):
                sl = slice(0, HF) if cols is None else cols
                n3 = nsb[:, sl].rearrange("p (h f) -> p h f", f=OUT_F)
                nh = n3.shape[1]
                h0 = sl.start // OUT_F
                r3 = _bcast_last(rc_ap[:, h0:h0 + nh], OUT_F)
                if ic in POOLFIN:
                    nf_sb = evp.tile([P, HF], f16, tag="evict")
                    nc.scalar.activation(nf_sb[:, sl], pF[:, sl], COPY)
                    nc.gpsimd.tensor_tensor(
                        n3, nf_sb[:, sl].rearrange("p (h f) -> p h f",
                                                   f=OUT_F), r3,
                        op=MULT)
                else:
                    p3 = pF[:, sl].rearrange("p (h f) -> p h f", f=OUT_F)
                    nc.vector.tensor_tensor(n3, p3, r3, op=MULT)

            # --- wave 0: projection interleaved with ics 0..WAVE-1.
            # Aggregation runs LAG j-chunks behind projection; pair 7 is
            # deferred past the wave boundary so PE has useful work (ic2's
            # pairs 0-6) while the last multiplies drain. ---
            for jc in range(DSTLEAD):
                dst_mm(jc)
            exp_pair(0)
            for step in range(JC + LAG):
                if step < JC:
                    proj(step)
                    if DSTLEAD and step + DSTLEAD < JC:
                        dst_mm(step + DSTLEAD)
                    if step % 2 == 1 and (step + 1) // 2 < JC // 2:
                        # dsts lead by DSTLEAD, so pair (step+1)//2 is
                        # complete: issue its exp a full pair-step early
                        exp_pair((step + 1) // 2)
                    if step in POOLJC:
                        mult_evict(step)
                    mult(step)
                    if step in (7, JC - 1):
                        e8_copy(step // 2)
                ready = step - LAG
                if ready >= 0 and ready % 2 == 1:
                    c = ready // 2
                    if c < JG - 1:
                        for k in range(WAVE):
                            mm_group(pFs[k], k, c)

            NW1 = IC - WAVE
            pF1 = []
            rc1 = scr.tile([P, NW1, H], f32, tag="rc1")
            nsb1 = nsbp.tile([P, NW1, HF], f16, tag="nsb", name="nsb1")

            def num_dr(pF, ic, c, start=None):
                lhs2 = adjT_pair(c)[:, :, ic * P:(ic + 1) * P]
                nc.tensor.matmul(
                    pF[:], lhs2, g_hi[:, 2 * c:2 * c + 2, :],
                    start=(c == 0) if start is None else start,
                    stop=(c == JG - 1),
                    perf_mode=DR, skip_group_check=True)

            def den_dr(ic, c):
                nc.tensor.matmul(
                    pden[:, ic, :],
                    adjT_pair(c)[:, :, ic * P:(ic + 1) * P],
                    e8_sb[:, 2 * c:2 * c + 2, :], start=False,
                    stop=(c == JG - 1),
                    perf_mode=DR, skip_group_check=True)

            # --- wave boundary: mult-independent PE work (ic2/ic3 pairs
            # 0-6, wave-1 dens) covers the latency of the last exp->multiply
            # chains before pair 7 is consumed ---
            NBIC = 2 if BIC3 else 1
            for kk in range(NBIC):
                pF = psh.tile([P, HF], f32, tag="hfeat", name=f"pF1_{kk}")
                pF1.append(pF)
                for c in range(JG - 1):
                    num_dr(pF, WAVE + kk, c)
            for ic in range(WAVE, IC):
                for c in range(BDENS):
                    den_dr(ic, c)
            # wave0 pair 7, then recip + fins + store
            for k in range(WAVE):
                mm_group(pFs[k], k, JG - 1)
            for ic in range(WAVE, IC):
                for c in range(BDENS, JG):
                    den_dr(ic, c)
            rc0 = scr.tile([P, WAVE, H], f32, tag="rc0")
            nc.vector.reciprocal(rc0[:], pden[:, 0:WAVE, :])
            nsb0 = nsbp.tile([P, WAVE, HF], f16, tag="nsb", name="nsb0")
            for k in range(WAVE):
                fin_mul(k, pFs[k], rc0[:, k, :], nsb0[:, k])
            nc.sync.dma_start(out_d[:, 0:WAVE * HF], nsb0[:])
            nc.vector.reciprocal(rc1[:], pden[:, WAVE:IC, :])
            for kk in range(NBIC):
                num_dr(pF1[kk], WAVE + kk, JG - 1, start=False)

            def fin_store1(k):
                """Finalize wave-1 ic WAVE+k and store per schedule."""
                fin_mul(WAVE + k, pF1[k], rc1[:, k, :], nsb1[:, k])
                if k in (1, 3):
                    nc.sync.dma_start(
                        out_d[:, (WAVE + k - 1) * HF:(WAVE + k + 1) * HF],
                        nsb1[:, k - 1:k + 1])
                elif k == 4:
                    nc.sync.dma_start(
                        out_d[:, (WAVE + k) * HF:(WAVE + k + 1) * HF],
                        nsb1[:, k])

            for k in range(NBIC, NW1 - 1):
                ic = WAVE + k
                if k < 2:
                    pF = psh.tile([P, HF], f32, tag="hfeat", name=f"pF1_{k}")
                else:
                    pF = psnum.tile([P, HF], f32, tag="num", name=f"pF1_{k}")
                pF1.append(pF)
                for c in range(JG):
                    num_dr(pF, ic, c)
                    if c == 3 and k - NBIC + 1 >= 1:
                        fin_store1(k - NBIC)
            # last ic: optionally COLUMN-split into separate psum tiles;
            # fin of the first half overlaps the second half's matmuls
            k = NW1 - 1
            if FIN7SPLIT:
                pF7 = [psnum.tile([P, HF // 2], f32, tag="num", name="pF7a"),
                       psh.tile([P, HF // 2], f32, tag="hfeat", name="pF7b")]
                pF1.append(pF7[0])
                for hh in range(2):
                    sl = slice(hh * (HF // 2), (hh + 1) * (HF // 2))
                    for c in range(JG):
                        lhs2 = adjT_pair(c)[:, :, (IC - 1) * P:IC * P]
                        nc.tensor.matmul(
                            pF7[hh][:], lhs2,
                            g_hi[:, 2 * c:2 * c + 2, sl],
                            start=(c == 0), stop=(c == JG - 1),
                            perf_mode=DR, skip_group_check=True)
                        if hh == 0 and c == 3:
                            fin_store1(k - 1)
                        elif hh == 0 and c == 5 and NBIC == 2:
                            fin_store1(k - NBIC)
                    n3 = nsb1[:, k, sl].rearrange("p (h f) -> p h f",
                                                  f=OUT_F)
                    r3 = _bcast_last(rc1[:, k, 4 * hh:4 * hh + 4], OUT_F)
                    p3 = pF7[hh][:].rearrange("p (h f) -> p h f", f=OUT_F)
                    nc.vector.tensor_tensor(n3, p3, r3, op=MULT)
            else:
                pF = psnum.tile([P, HF], f32, tag="num", name="pF7")
                pF1.append(pF)
                for c in range(JG):
                    num_dr(pF, IC - 1, c)
                    if c == 3:
                        fin_store1(k - 1)
                    elif c == 5 and NBIC == 2:
                        fin_store1(k - NBIC)
                fin_mul(IC - 1, pF, rc1[:, k, :], nsb1[:, k])
            nc.sync.dma_start(
                out_d[:, (IC - 1) * HF:IC * HF], nsb1[:, k])

    nc.compile()
    return nc


def _get_nc():
    if "nc" not in _CACHE:
        _CACHE["nc"] = _build()
    return _CACHE["nc"]


def _make_in_maps(x, adj, weight, attn_dst):
    x = np.ascontiguousarray(np.asarray(x), dtype=np.float32)
    adj = np.asarray(adj)
    weight = np.ascontiguousarray(np.asarray(weight), dtype=np.float32)
    attn_dst = np.ascontiguousarray(np.asarray(attn_dst), dtype=np.float32)

    # fold attn_dst into the weight: wdst[k, h] = sum_f W[k, h*64+f]*adst[h, f]
    wdst = (weight.reshape(IN_F, H, OUT_F) * attn_dst[None]).sum(-1)

    # pack [W | wdst] -> [P, KC, HF+H]
    wfull = np.concatenate([weight, wdst], axis=1)        # [IN_F, WD]
    w_kp = np.ascontiguousarray(
        wfull.reshape(KC, P, WD).transpose(1, 0, 2).reshape(P, KC * WD)
    ).astype(F16)

    in_maps = []
    for core in range(NCORES):
        b = core // 2
        half = core % 2
        # xt layout [p, jc, kc, j']: x[b][jc*128 + j', kc*128 + p]
        xt = x[b].T.reshape(KC, P, JC, P)              # [kc, p, jc, j']
        xt_kp = np.ascontiguousarray(
            xt.transpose(1, 2, 0, 3).reshape(P, JC * KC * P)
        ).astype(F16)
        adjt = adj[b].T[:, half * ROWS:(half + 1) * ROWS]  # [N, ROWS]
        in_maps.append({
            "adjt": np.ascontiguousarray(adjt, dtype=np.float32).astype(FP8),
            "w": w_kp,
            "xt": xt_kp,
        })
    return in_maps


def _run_device(in_maps):
    from concourse import bass_utils

    nc = _get_nc()
    res = bass_utils.run_bass_kernel_spmd(
        nc, in_maps, core_ids=list(range(NCORES)))
    return [dict(r) for r in res.results]


def _run_device_subprocess(in_maps):
    """Fresh-process fallback: a wedged accelerator surfaces as
    NRT_EXEC_UNIT_UNRECOVERABLE and poisons the in-process PJRT client;
    a new process gets a fresh axon session and a reset device."""
    import os
    import pickle
    import subprocess
    import sys
    import tempfile

    d = tempfile.mkdtemp(prefix="gat_kernel_")
    inp = os.path.join(d, "in.pkl")
    outp = os.path.join(d, "out.pkl")
    with open(inp, "wb") as f:
        pickle.dump(in_maps, f)
    code = (
        "import pickle, sys\n"
        f"sys.path.insert(0, {os.path.dirname(os.path.abspath(__file__))!r})\n"
        "import kernel\n"
        f"in_maps = pickle.load(open({inp!r}, 'rb'))\n"
        f"pickle.dump(kernel._run_device(in_maps), open({outp!r}, 'wb'))\n"
    )
    env = dict(os.environ, GAT_KERNEL_SUBPROC="1")
    subprocess.run([sys.executable, "-c", code], check=True, env=env,
                   timeout=1200)
    with open(outp, "rb") as f:
        return pickle.load(f)


def kernel(x, adj, weight, attn_src, attn_dst):
    import os
    import time

    in_maps = _make_in_maps(x, adj, weight, attn_dst)
    try:
        results = _run_device(in_maps)
    except Exception:
        if os.environ.get("GAT_KERNEL_SUBPROC") == "1":
            raise
        time.sleep(2)
        results = _run_device_subprocess(in_maps)

    out = np.empty((B, N, HF), dtype=np.float32)
    for core in range(NCORES):
        b = core // 2
        half = core % 2
        res = results[core]["out"].astype(np.float32)      # [P, IC*HF]
        for q in range(IC):
            r0 = half * ROWS + q * P
            out[b, r0:r0 + P, :] = res[:, q * HF:(q + 1) * HF]
    return out


# revision 42
# speedup vs baseline: 1.0008x; 1.0008x over previous
"""GAT layer kernel for Trainium2, 8 NeuronCores.

Problem: nn_GATLayer (B=4, N=2048, IN_F=256, OUT_F=64, H=8).

Key algebra: softmax over j of (src[b,i,h] + dst[b,j,h]) masked by adj[b,i,j].
src[b,i,h] is constant over j, so it cancels in the softmax:
    out[b,i,(h,f)] = (adj[b,i,:] @ g[b,:, (h,f)]) / (adj[b,i,:] @ e[b,:,h])
with hfeat = x@W (per-head features), dst[j,h] = x[j,:] @ (W . attn_dst)[:,h],
e = exp(dst), g = e * hfeat.  attn_src is mathematically irrelevant.

Sharding: 8 cores = 4 batches x 2 row-halves of i (softmax is over j only,
so row-sharding of i needs no communication).

Numerics/perf design (final):
  - fp16 projection (x, W, wdst in fp16; psum f32).  wdst = W . attn_dst
    folded on host and packed into the same dram tensor as W (one DMA).
  - e = exp(dst + ln(3/16)) in fp16; the global 3/16 scale cancels in the
    softmax ratio and keeps g inside e4m3 range.
  - g = e*hfeat written DIRECTLY as fp8 e4m3 by the multiply; the numerator
    is a SINGLE fp8 DoubleRow pass (no hi/lo split, no cast/subtract
    pipeline).  Measured absmax-rel err ~1.5e-2 < 2e-2 gate.
  - Wave-0 denominators are plain f16 matmuls inside the aggregation loop;
    wave-1 denominators are DoubleRow matmuls on an fp8 copy of e and are
    hoisted (with one batched reciprocal) off the finalize tail chains.
  - dst matmuls are issued DSTLEAD j-chunks ahead of the hfeat matmuls so
    the exp's PSUM read never blocks the next chunk's PSUM write (the dep
    tracker is tile-granular).
  - Elementwise per j-chunk: exp batched per pair (ACT); the e*hfeat
    multiply runs on DVE straight from PSUM for most chunks, and via
    ACT-evict(psum->fp16 sbuf) + Pool multiply for POOLJC chunks.
    Finalize (num * 1/den) likewise DVE or ACT+Pool per POOLFIN.
  - Two waves of i-chunks: 2 interleaved with the projection (aggregation
    LAG j-chunks behind), 6 after, completions staggered so fins/stores
    drain progressively; the last i-chunk is column-split into two psum
    tiles so its first fin overlaps its second half's matmuls.
  - 8 input DMAs total (w, 3 xt slabs, 4 adjT groups) keep the shared
    HWDGE/DMA engines ahead of consumption; PE pre-warm dummy matmuls pin
    pe_busy_start early so the p-state ramp completes behind the input
    DMA latency.
"""

import numpy as np
import ml_dtypes

B, N, IN_F, OUT_F, H = 4, 2048, 256, 64, 8
HF = H * OUT_F            # 512 concat features
NCORES = 8
ROWS = B * N // NCORES    # 1024 destination rows per core
P = 128
IC = ROWS // P            # 8 i-chunks per core
JC = N // P               # 16 j-chunks
KC = IN_F // P            # 2 k-chunks
JG = 8                    # DoubleRow j-chunk pairs
AG = 4                    # adjT DMA groups (4 j-chunks each)
WAVE = 2                  # i-chunks in the first (interleaved) wave
NDUMMY = 2                # PE p-state pre-warm matmuls (sets pe_busy_start)
LAG = 8                   # j-chunks the aggregation trails projection by
ESCALE = 0.1875           # global scale on e; cancels in the softmax ratio
WD = HF + H               # packed W columns (hfeat + dst)

POOLJC = (2, 4, 6, 9, 11)       # j-chunks whose multiply goes ACT-evict+Pool
DSTLEAD = 2               # j-chunks the dst matmuls lead the hfeat matmuls by
BDENS = 4                 # wave-1 den pairs issued before wave0 pair 7
BIC3 = False              # ic3's pairs 0-6 run at the wave boundary
POOLFIN = (0, 2)                # i-chunks whose finalize goes ACT-evict+Pool
# xt DMA slabs (j-chunk ranges) interleaved with adjT groups:
#   w, s0, s1, g0, s2, g1, g2, g3
XSLABS = ((0, 2), (2, 8), (8, 16))
FIN7SPLIT = True          # column-split the last ic across two psum tiles

F16 = np.float16
FP8 = ml_dtypes.float8_e4m3

_CACHE = {}


def _bcast_last(ap, n):
    """View ap with an extra innermost broadcast (stride-0) dim of size n."""
    ap2 = ap.unsqueeze(len(ap.shape))
    return ap2.broadcast_to(tuple(ap.shape) + (n,))


def _build():
    import concourse.mybir as mybir
    import concourse.tile as tile
    from concourse import bacc

    f32 = mybir.dt.float32
    f16 = mybir.dt.float16
    fp8 = mybir.dt.float8e4
    MULT = mybir.AluOpType.mult
    DR = mybir.MatmulPerfMode.DoubleRow
    COPY = mybir.ActivationFunctionType.Copy
    EXP = mybir.ActivationFunctionType.Exp

    nc = bacc.Bacc(trn_type="TRN2", debug=False, target_bir_lowering=False)

    adjt_d = nc.dram_tensor("adjt", [N, ROWS], fp8, kind="ExternalInput")
    w_d = nc.dram_tensor("w", [P, KC * WD], f16, kind="ExternalInput")
    xt_d = nc.dram_tensor("xt", [P, JC * KC * P], f16, kind="ExternalInput")
    out_d = nc.dram_tensor("out", [P, IC * HF], f16, kind="ExternalOutput")

    with tile.TileContext(nc) as tc:
        with (
            tc.tile_pool(name="setup", bufs=1) as setup,
            tc.tile_pool(name="gpool", bufs=1) as gpool,
            tc.tile_pool(name="scratch", bufs=2) as scr,
            tc.tile_pool(name="evict", bufs=3) as evp,
            tc.tile_pool(name="adjT", bufs=1) as adjTp,
            tc.tile_pool(name="warm", bufs=1) as warmp,
            tc.tile_pool(name="ps_num", bufs=WAVE, space="PSUM") as psnum,
            tc.tile_pool(name="ps_h", bufs=5, space="PSUM") as psh,
            tc.tile_pool(name="ps_dd", bufs=1, space="PSUM") as psdd,
            tc.tile_pool(name="nsbp", bufs=2) as nsbp,
        ):
            # --- PE pre-warm: dummies into pF0; its first real matmul
            # re-starts the accumulation group, so no extra bank is used ---
            pFs = [psnum.tile([P, HF], f32, tag="num", name=f"pF0_{k}")
                   for k in range(WAVE)]
            junk = warmp.tile([P, 1], f16)
            nc.vector.memset(junk[:], 0.0)
            ebias = warmp.tile([P, 1], f32)
            nc.vector.memset(ebias[:], float(np.log(ESCALE)))
            junk_l = junk[:].broadcast_to((P, P))
            junk_rhs = junk[:].unsqueeze(1).broadcast_to((P, 4, P))
            for _ in range(NDUMMY):
                nc.tensor.matmul(pFs[0][:], junk_l, junk_rhs,
                                 start=True, stop=True, skip_group_check=True)

            # --- input streams via HWDGE (sync), EDF-ordered ---
            w_sb = setup.tile([P, KC, WD], f16)
            w_kc = [w_sb[:, kc] for kc in range(KC)]
            xT_sb = setup.tile([P, JC, KC, P], f16)
            adjT_g = []
            for G in range(AG):
                adjT_g.append(adjTp.tile([P, 4, ROWS], fp8, tag=f"adjt{G}",
                                         name=f"adjt{G}"))

            w_v = w_d.rearrange("p (kc n) -> p kc n", kc=KC)
            xt_v = xt_d.rearrange("p (jc kc j) -> p jc kc j", jc=JC, kc=KC)

            def load_slab(s):
                j0, j1 = XSLABS[s]
                nc.sync.dma_start(
                    xT_sb[:, j0:j1], xt_v[:, j0:j1])

            def load_adjt(G):
                nc.sync.dma_start(
                    adjT_g[G][:],
                    adjt_d[G * 4 * P:(G + 1) * 4 * P, :].rearrange(
                        "(jc jp) i -> jp jc i", jp=P),
                )

            def adjT_pair(c):
                # [P, 2, ROWS] view of j-chunk pair c
                t = 2 * (c % 2)
                return adjT_g[c // 2][:, t:t + 2]

            nc.sync.dma_start(w_sb[:], w_v[:])
            load_slab(0)
            load_slab(1)
            load_adjt(0)
            load_slab(2)
            for G in range(1, AG):
                load_adjt(G)

            def xT(kc, jc):
                return xT_sb[:, jc, kc, :]

            g_hi = gpool.tile([P, JC, HF], fp8)
            e_sb = gpool.tile([P, JC, H], f16)
            e8_sb = gpool.tile([P, JC, H], fp8)
            # dst logits AND denominators share ONE psum bank: dst's single
            # start=True marks the whole bank pending-zero, so den matmuls
            # accumulate with start=False onto zeroed bytes
            pdd = psdd.tile([P, JC + IC, H], f32)
            pden = pdd[:, JC:JC + IC, :]

            def pdst(jc):
                return pdd[:, jc, :]

            phs = {}

            def dst_mm(jc):
                for kc in range(KC):
                    nc.tensor.matmul(
                        pdst(jc), xT(kc, jc), w_kc[kc][:, HF:WD],
                        start=(jc == 0 and kc == 0), stop=(kc == KC - 1),
                        skip_group_check=True,
                    )

            def proj(jc):
                if DSTLEAD == 0:
                    dst_mm(jc)
                ph = psh.tile([P, HF], f32, tag="hfeat")
                phs[jc] = ph
                for kc in range(KC):
                    nc.tensor.matmul(
                        ph[:], xT(kc, jc), w_kc[kc][:, 0:HF],
                        start=(kc == 0), stop=(kc == KC - 1),
                    )

            def exp_pair(c):
                # exp for j-chunks 2c, 2c+1 in one ACT instruction
                nc.scalar.activation(
                    e_sb[:, 2 * c:2 * c + 2, :], pdd[:, 2 * c:2 * c + 2, :],
                    EXP, bias=ebias[:],
                )

            def e8_copy(c):
                # batched fp8 half-copies of e for the wave-1 DoubleRow
                # denominator matmuls (Pool; off the DVE critical path)
                h = slice(0, 8) if c == 3 else slice(8, JC)
                nc.gpsimd.tensor_copy(e8_sb[:, h], e_sb[:, h])

            hfs = {}

            def mult_evict(jc):
                # eager psum->sbuf eviction for Pool-path chunks: frees the
                # ph bank as soon as possible (ACT, ahead of later exps)
                ph = phs.pop(jc)
                hf_sb = evp.tile([P, HF], f16, tag="evict")
                nc.scalar.activation(hf_sb[:], ph[:], COPY)
                hfs[jc] = hf_sb

            def mult(jc):
                # g_hi[jc] = e[jc] (bcast over F) * hfeat[jc], fp8 out
                e3 = _bcast_last(e_sb[:, jc, :], OUT_F)
                g3 = g_hi[:, jc, :].rearrange("p (h f) -> p h f", h=H)
                if jc in POOLJC:
                    hf_sb = hfs.pop(jc)
                    nc.gpsimd.tensor_tensor(
                        g3, hf_sb[:].rearrange("p (h f) -> p h f", h=H), e3,
                        op=MULT)
                else:
                    h3 = phs.pop(jc)[:].rearrange("p (h f) -> p h f", h=H)
                    nc.vector.tensor_tensor(g3, h3, e3, op=MULT)

            def mm_group(pF, ic, c):
                """Single-pass DoubleRow numerator and f16 denominator for
                j-chunk pair c (j-chunks 2c, 2c+1) into i-chunk ic."""
                ap = adjT_pair(c)
                nc.tensor.matmul(
                    pF[:], ap[:, :, ic * P:(ic + 1) * P],
                    g_hi[:, 2 * c:2 * c + 2, :],
                    start=(c == 0), stop=(c == JG - 1),
                    perf_mode=DR, skip_group_check=True,
                )
                for t in range(2):
                    jc = 2 * c + t
                    nc.tensor.matmul(
                        pden[:, ic, :], ap[:, t, ic * P:(ic + 1) * P],
                        e_sb[:, jc, :],
                        start=False, stop=(jc == JC - 1),
                        skip_group_check=True,
                    )

            def fin_mul(ic, pF, rc_ap, nsb, cols=None):
                sl = slice(0, HF) if cols is None else cols
                n3 = nsb[:, sl].rearrange("p (h f) -> p h f", f=OUT_F)
                nh = n3.shape[1]
                h0 = sl.start // OUT_F
                r3 = _bcast_last(rc_ap[:, h0:h0 + nh], OUT_F)
                if ic in POOLFIN:
                    nf_sb = evp.tile([P, HF], f16, tag="evict")
                    nc.scalar.activation(nf_sb[:, sl], pF[:, sl], COPY)
                    nc.gpsimd.tensor_tensor(
                        n3, nf_sb[:, sl].rearrange("p (h f) -> p h f",
                                                   f=OUT_F), r3,
                        op=MULT)
                else:
                    p3 = pF[:, sl].rearrange("p (h f) -> p h f", f=OUT_F)
                    nc.vector.tensor_tensor(n3, p3, r3, op=MULT)

            # --- wave 0: projection interleaved with ics 0..WAVE-1.
            # Aggregation runs LAG j-chunks behind projection; pair 7 is
            # deferred past the wave boundary so PE has useful work (ic2's
            # pairs 0-6) while the last multiplies drain. ---
            for jc in range(DSTLEAD):
                dst_mm(jc)
            exp_pair(0)
            for step in range(JC + LAG):
                if step < JC:
                    proj(step)
                    if DSTLEAD and step + DSTLEAD < JC:
                        dst_mm(step + DSTLEAD)
                    if step % 2 == 1 and (step + 1) // 2 < JC // 2:
                        # dsts lead by DSTLEAD, so pair (step+1)//2 is
                        # complete: issue its exp a full pair-step early
                        exp_pair((step + 1) // 2)
                    if step in POOLJC:
                        mult_evict(step)
                    mult(step)
                    if step in (7, JC - 1):
                        e8_copy(step // 2)
                ready = step - LAG
                if ready >= 0 and ready % 2 == 1:
                    c = ready // 2
                    if c < JG - 1:
                        for k in range(WAVE):
                            mm_group(pFs[k], k, c)

            NW1 = IC - WAVE
            pF1 = []
            rc1 = scr.tile([P, NW1, H], f32, tag="rc1")
            nsb1 = nsbp.tile([P, NW1, HF], f16, tag="nsb", name="nsb1")

            def num_dr(pF, ic, c, start=None):
                lhs2 = adjT_pair(c)[:, :, ic * P:(ic + 1) * P]
                nc.tensor.matmul(
                    pF[:], lhs2, g_hi[:, 2 * c:2 * c + 2, :],
                    start=(c == 0) if start is None else start,
                    stop=(c == JG - 1),
                    perf_mode=DR, skip_group_check=True)

            def den_dr(ic, c):
                nc.tensor.matmul(
                    pden[:, ic, :],
                    adjT_pair(c)[:, :, ic * P:(ic + 1) * P],
                    e8_sb[:, 2 * c:2 * c + 2, :], start=False,
                    stop=(c == JG - 1),
                    perf_mode=DR, skip_group_check=True)

            # --- wave boundary: mult-independent PE work (ic2/ic3 pairs
            # 0-6, wave-1 dens) covers the latency of the last exp->multiply
            # chains before pair 7 is consumed ---
            NBIC = 2 if BIC3 else 1
            for kk in range(NBIC):
                pF = psh.tile([P, HF], f32, tag="hfeat", name=f"pF1_{kk}")
                pF1.append(pF)
                for c in range(JG - 1):
                    num_dr(pF, WAVE + kk, c)
            for ic in range(WAVE, IC):
                for c in range(BDENS):
                    den_dr(ic, c)
            # wave0 pair 7, then recip + fins + store
            for k in range(WAVE):
                mm_group(pFs[k], k, JG - 1)
            for ic in range(WAVE, IC):
                for c in range(BDENS, JG):
                    den_dr(ic, c)
            rc0 = scr.tile([P, WAVE, H], f32, tag="rc0")
            nc.vector.reciprocal(rc0[:], pden[:, 0:WAVE, :])
            nsb0 = nsbp.tile([P, WAVE, HF], f16, tag="nsb", name="nsb0")
            for k in range(WAVE):
                fin_mul(k, pFs[k], rc0[:, k, :], nsb0[:, k])
            nc.sync.dma_start(out_d[:, 0:WAVE * HF], nsb0[:])
            nc.vector.reciprocal(rc1[:], pden[:, WAVE:IC, :])
            for kk in range(NBIC):
                num_dr(pF1[kk], WAVE + kk, JG - 1, start=False)

            def fin_store1(k):
                """Finalize wave-1 ic WAVE+k and store per schedule."""
                fin_mul(WAVE + k, pF1[k], rc1[:, k, :], nsb1[:, k])
                if k in (1, 3):
                    nc.sync.dma_start(
                        out_d[:, (WAVE + k - 1) * HF:(WAVE + k + 1) * HF],
                        nsb1[:, k - 1:k + 1])
                elif k == 4:
                    nc.sync.dma_start(
                        out_d[:, (WAVE + k) * HF:(WAVE + k + 1) * HF],
                        nsb1[:, k])

            for k in range(NBIC, NW1 - 1):
                ic = WAVE + k
                if k < 2:
                    pF = psh.tile([P, HF], f32, tag="hfeat", name=f"pF1_{k}")
                else:
                    pF = psnum.tile([P, HF], f32, tag="num", name=f"pF1_{k}")
                pF1.append(pF)
                for c in range(JG):
                    num_dr(pF, ic, c)
                    if c == 3 and k - NBIC + 1 >= 1:
                        fin_store1(k - NBIC)
            # last ic: optionally COLUMN-split into separate psum tiles;
            # fin of the first half overlaps the second half's matmuls
            k = NW1 - 1
            if FIN7SPLIT:
                pF7 = [psnum.tile([P, HF // 2], f32, tag="num", name="pF7a"),
                       psh.tile([P, HF // 2], f32, tag="hfeat", name="pF7b")]
                pF1.append(pF7[0])
                for hh in range(2):
                    sl = slice(hh * (HF // 2), (hh + 1) * (HF // 2))
                    for c in range(JG):
                        lhs2 = adjT_pair(c)[:, :, (IC - 1) * P:IC * P]
                        nc.tensor.matmul(
                            pF7[hh][:], lhs2,
                            g_hi[:, 2 * c:2 * c + 2, sl],
                            start=(c == 0), stop=(c == JG - 1),
                            perf_mode=DR, skip_group_check=True)
                        if hh == 0 and c == 3:
                            fin_store1(k - 1)
                        elif hh == 0 and c == 5 and NBIC == 2:
                            fin_store1(k - NBIC)
                    n3 = nsb1[:, k, sl].rearrange("p (h f) -> p h f",
                                                  f=OUT_F)
                    r3 = _bcast_last(rc1[:, k, 4 * hh:4 * hh + 4], OUT_F)
                    p3 = pF7[hh][:].rearrange("p (h f) -> p h f", f=OUT_F)
                    nc.vector.tensor_tensor(n3, p3, r3, op=MULT)
            else:
                pF = psnum.tile([P, HF], f32, tag="num", name="pF7")
                pF1.append(pF)
                for c in range(JG):
                    num_dr(pF, IC - 1, c)
                    if c == 3:
                        fin_store1(k - 1)
                    elif c == 5 and NBIC == 2:
                        fin_store1(k - NBIC)
                fin_mul(IC - 1, pF, rc1[:, k, :], nsb1[:, k])
            nc.sync.dma_start(
                out_d[:, (IC - 1) * HF:IC * HF], nsb1[:, k])

    nc.compile()
    return nc


def _get_nc():
    if "nc" not in _CACHE:
        _CACHE["nc"] = _build()
    return _CACHE["nc"]


def _make_in_maps(x, adj, weight, attn_dst):
    x = np.ascontiguousarray(np.asarray(x), dtype=np.float32)
    adj = np.asarray(adj)
    weight = np.ascontiguousarray(np.asarray(weight), dtype=np.float32)
    attn_dst = np.ascontiguousarray(np.asarray(attn_dst), dtype=np.float32)

    # fold attn_dst into the weight: wdst[k, h] = sum_f W[k, h*64+f]*adst[h, f]
    wdst = (weight.reshape(IN_F, H, OUT_F) * attn_dst[None]).sum(-1)

    # pack [W | wdst] -> [P, KC, HF+H]
    wfull = np.concatenate([weight, wdst], axis=1)        # [IN_F, WD]
    w_kp = np.ascontiguousarray(
        wfull.reshape(KC, P, WD).transpose(1, 0, 2).reshape(P, KC * WD)
    ).astype(F16)

    in_maps = []
    for core in range(NCORES):
        b = core // 2
        half = core % 2
        # xt layout [p, jc, kc, j']: x[b][jc*128 + j', kc*128 + p]
        xt = x[b].T.reshape(KC, P, JC, P)              # [kc, p, jc, j']
        xt_kp = np.ascontiguousarray(
            xt.transpose(1, 2, 0, 3).reshape(P, JC * KC * P)
        ).astype(F16)
        adjt = adj[b].T[:, half * ROWS:(half + 1) * ROWS]  # [N, ROWS]
        in_maps.append({
            "adjt": np.ascontiguousarray(adjt, dtype=np.float32).astype(FP8),
            "w": w_kp,
            "xt": xt_kp,
        })
    return in_maps


def _run_device(in_maps):
    from concourse import bass_utils

    nc = _get_nc()
    res = bass_utils.run_bass_kernel_spmd(
        nc, in_maps, core_ids=list(range(NCORES)))
    return [dict(r) for r in res.results]


def _run_device_subprocess(in_maps):
    """Fresh-process fallback: a wedged accelerator surfaces as
    NRT_EXEC_UNIT_UNRECOVERABLE and poisons the in-process PJRT client;
    a new process gets a fresh axon session and a reset device."""
    import os
    import pickle
    import subprocess
    import sys
    import tempfile

    d = tempfile.mkdtemp(prefix="gat_kernel_")
    inp = os.path.join(d, "in.pkl")
    outp = os.path.join(d, "out.pkl")
    with open(inp, "wb") as f:
        pickle.dump(in_maps, f)
    code = (
        "import pickle, sys\n"
        f"sys.path.insert(0, {os.path.dirname(os.path.abspath(__file__))!r})\n"
        "import kernel\n"
        f"in_maps = pickle.load(open({inp!r}, 'rb'))\n"
        f"pickle.dump(kernel._run_device(in_maps), open({outp!r}, 'wb'))\n"
    )
    env = dict(os.environ, GAT_KERNEL_SUBPROC="1")
    subprocess.run([sys.executable, "-c", code], check=True, env=env,
                   timeout=1200)
    with open(outp, "rb") as f:
        return pickle.load(f)


def kernel(x, adj, weight, attn_src, attn_dst):
    import os
    import time

    in_maps = _make_in_maps(x, adj, weight, attn_dst)
    try:
        results = _run_device(in_maps)
    except Exception:
        if os.environ.get("GAT_KERNEL_SUBPROC") == "1":
            raise
        time.sleep(2)
        results = _run_device_subprocess(in_maps)

    out = np.empty((B, N, HF), dtype=np.float32)
    for core in range(NCORES):
        b = core // 2
        half = core % 2
        res = results[core]["out"].astype(np.float32)      # [P, IC*HF]
        for q in range(IC):
            r0 = half * ROWS + q * P
            out[b, r0:r0 + P, :] = res[:, q * HF:(q + 1) * HF]
    return out


# revision 43
# speedup vs baseline: 1.0016x; 1.0008x over previous
"""GAT layer kernel for Trainium2, 8 NeuronCores.

Problem: nn_GATLayer (B=4, N=2048, IN_F=256, OUT_F=64, H=8).

Key algebra: softmax over j of (src[b,i,h] + dst[b,j,h]) masked by adj[b,i,j].
src[b,i,h] is constant over j, so it cancels in the softmax:
    out[b,i,(h,f)] = (adj[b,i,:] @ g[b,:, (h,f)]) / (adj[b,i,:] @ e[b,:,h])
with hfeat = x@W (per-head features), dst[j,h] = x[j,:] @ (W . attn_dst)[:,h],
e = exp(dst), g = e * hfeat.  attn_src is mathematically irrelevant.

Sharding: 8 cores = 4 batches x 2 row-halves of i (softmax is over j only,
so row-sharding of i needs no communication).

Numerics/perf design (final):
  - fp16 projection (x, W, wdst in fp16; psum f32).  wdst = W . attn_dst
    folded on host and packed into the same dram tensor as W (one DMA).
  - e = exp(dst + ln(3/16)) in fp16; the global 3/16 scale cancels in the
    softmax ratio and keeps g inside e4m3 range.
  - g = e*hfeat written DIRECTLY as fp8 e4m3 by the multiply; the numerator
    is a SINGLE fp8 DoubleRow pass (no hi/lo split, no cast/subtract
    pipeline).  Measured absmax-rel err ~1.5e-2 < 2e-2 gate.
  - Wave-0 denominators are plain f16 matmuls inside the aggregation loop;
    wave-1 denominators are DoubleRow matmuls on an fp8 copy of e and are
    hoisted (with one batched reciprocal) off the finalize tail chains.
  - dst matmuls are issued DSTLEAD j-chunks ahead of the hfeat matmuls so
    the exp's PSUM read never blocks the next chunk's PSUM write (the dep
    tracker is tile-granular).
  - Elementwise per j-chunk: exp batched per pair (ACT); the e*hfeat
    multiply runs on DVE straight from PSUM for most chunks, and via
    ACT-evict(psum->fp16 sbuf) + Pool multiply for POOLJC chunks.
    Finalize (num * 1/den) likewise DVE or ACT+Pool per POOLFIN.
  - Two waves of i-chunks: 2 interleaved with the projection (aggregation
    LAG j-chunks behind), 6 after, completions staggered so fins/stores
    drain progressively; the last i-chunk is column-split into two psum
    tiles so its first fin overlaps its second half's matmuls.
  - 8 input DMAs total (w, 3 xt slabs, 4 adjT groups) keep the shared
    HWDGE/DMA engines ahead of consumption; PE pre-warm dummy matmuls pin
    pe_busy_start early so the p-state ramp completes behind the input
    DMA latency.
"""

import numpy as np
import ml_dtypes

B, N, IN_F, OUT_F, H = 4, 2048, 256, 64, 8
HF = H * OUT_F            # 512 concat features
NCORES = 8
ROWS = B * N // NCORES    # 1024 destination rows per core
P = 128
IC = ROWS // P            # 8 i-chunks per core
JC = N // P               # 16 j-chunks
KC = IN_F // P            # 2 k-chunks
JG = 8                    # DoubleRow j-chunk pairs
AG = 4                    # adjT DMA groups (4 j-chunks each)
WAVE = 2                  # i-chunks in the first (interleaved) wave
NDUMMY = 2                # PE p-state pre-warm matmuls (sets pe_busy_start)
LAG = 9                   # j-chunks the aggregation trails projection by
ESCALE = 0.1875           # global scale on e; cancels in the softmax ratio
WD = HF + H               # packed W columns (hfeat + dst)

POOLJC = (2, 4, 6, 9, 11)       # j-chunks whose multiply goes ACT-evict+Pool
DSTLEAD = 2               # j-chunks the dst matmuls lead the hfeat matmuls by
BDENS = 4                 # wave-1 den pairs issued before wave0 pair 7
BIC3 = False              # ic3's pairs 0-6 run at the wave boundary
POOLFIN = (0, 2)                # i-chunks whose finalize goes ACT-evict+Pool
# xt DMA slabs (j-chunk ranges) interleaved with adjT groups:
#   w, s0, s1, g0, s2, g1, g2, g3
XSLABS = ((0, 2), (2, 8), (8, 16))
FIN7SPLIT = True          # column-split the last ic across two psum tiles

F16 = np.float16
FP8 = ml_dtypes.float8_e4m3

_CACHE = {}


def _bcast_last(ap, n):
    """View ap with an extra innermost broadcast (stride-0) dim of size n."""
    ap2 = ap.unsqueeze(len(ap.shape))
    return ap2.broadcast_to(tuple(ap.shape) + (n,))


def _build():
    import concourse.mybir as mybir
    import concourse.tile as tile
    from concourse import bacc

    f32 = mybir.dt.float32
    f16 = mybir.dt.float16
    fp8 = mybir.dt.float8e4
    MULT = mybir.AluOpType.mult
    DR = mybir.MatmulPerfMode.DoubleRow
    COPY = mybir.ActivationFunctionType.Copy
    EXP = mybir.ActivationFunctionType.Exp

    nc = bacc.Bacc(trn_type="TRN2", debug=False, target_bir_lowering=False)

    adjt_d = nc.dram_tensor("adjt", [N, ROWS], fp8, kind="ExternalInput")
    w_d = nc.dram_tensor("w", [P, KC * WD], f16, kind="ExternalInput")
    xt_d = nc.dram_tensor("xt", [P, JC * KC * P], f16, kind="ExternalInput")
    out_d = nc.dram_tensor("out", [P, IC * HF], f16, kind="ExternalOutput")

    with tile.TileContext(nc) as tc:
        with (
            tc.tile_pool(name="setup", bufs=1) as setup,
            tc.tile_pool(name="gpool", bufs=1) as gpool,
            tc.tile_pool(name="scratch", bufs=2) as scr,
            tc.tile_pool(name="evict", bufs=4) as evp,
            tc.tile_pool(name="adjT", bufs=1) as adjTp,
            tc.tile_pool(name="warm", bufs=1) as warmp,
            tc.tile_pool(name="ps_num", bufs=WAVE, space="PSUM") as psnum,
            tc.tile_pool(name="ps_h", bufs=5, space="PSUM") as psh,
            tc.tile_pool(name="ps_dd", bufs=1, space="PSUM") as psdd,
            tc.tile_pool(name="nsbp", bufs=2) as nsbp,
        ):
            # --- PE pre-warm: dummies into pF0; its first real matmul
            # re-starts the accumulation group, so no extra bank is used ---
            pFs = [psnum.tile([P, HF], f32, tag="num", name=f"pF0_{k}")
                   for k in range(WAVE)]
            junk = warmp.tile([P, 1], f16)
            nc.vector.memset(junk[:], 0.0)
            ebias = warmp.tile([P, 1], f32)
            nc.vector.memset(ebias[:], float(np.log(ESCALE)))
            junk_l = junk[:].broadcast_to((P, P))
            junk_rhs = junk[:].unsqueeze(1).broadcast_to((P, 4, P))
            for _ in range(NDUMMY):
                nc.tensor.matmul(pFs[0][:], junk_l, junk_rhs,
                                 start=True, stop=True, skip_group_check=True)

            # --- input streams via HWDGE (sync), EDF-ordered ---
            w_sb = setup.tile([P, KC, WD], f16)
            w_kc = [w_sb[:, kc] for kc in range(KC)]
            xT_sb = setup.tile([P, JC, KC, P], f16)
            adjT_g = []
            for G in range(AG):
                adjT_g.append(adjTp.tile([P, 4, ROWS], fp8, tag=f"adjt{G}",
                                         name=f"adjt{G}"))

            w_v = w_d.rearrange("p (kc n) -> p kc n", kc=KC)
            xt_v = xt_d.rearrange("p (jc kc j) -> p jc kc j", jc=JC, kc=KC)

            def load_slab(s):
                j0, j1 = XSLABS[s]
                nc.sync.dma_start(
                    xT_sb[:, j0:j1], xt_v[:, j0:j1])

            def load_adjt(G):
                nc.sync.dma_start(
                    adjT_g[G][:],
                    adjt_d[G * 4 * P:(G + 1) * 4 * P, :].rearrange(
                        "(jc jp) i -> jp jc i", jp=P),
                )

            def adjT_pair(c):
                # [P, 2, ROWS] view of j-chunk pair c
                t = 2 * (c % 2)
                return adjT_g[c // 2][:, t:t + 2]

            nc.sync.dma_start(w_sb[:], w_v[:])
            load_slab(0)
            load_slab(1)
            load_adjt(0)
            load_slab(2)
            for G in range(1, AG):
                load_adjt(G)

            def xT(kc, jc):
                return xT_sb[:, jc, kc, :]

            g_hi = gpool.tile([P, JC, HF], fp8)
            e_sb = gpool.tile([P, JC, H], f16)
            e8_sb = gpool.tile([P, JC, H], fp8)
            # dst logits AND denominators share ONE psum bank: dst's single
            # start=True marks the whole bank pending-zero, so den matmuls
            # accumulate with start=False onto zeroed bytes
            pdd = psdd.tile([P, JC + IC, H], f32)
            pden = pdd[:, JC:JC + IC, :]

            def pdst(jc):
                return pdd[:, jc, :]

            phs = {}

            def dst_mm(jc):
                for kc in range(KC):
                    nc.tensor.matmul(
                        pdst(jc), xT(kc, jc), w_kc[kc][:, HF:WD],
                        start=(jc == 0 and kc == 0), stop=(kc == KC - 1),
                        skip_group_check=True,
                    )

            def proj(jc):
                if DSTLEAD == 0:
                    dst_mm(jc)
                ph = psh.tile([P, HF], f32, tag="hfeat")
                phs[jc] = ph
                for kc in range(KC):
                    nc.tensor.matmul(
                        ph[:], xT(kc, jc), w_kc[kc][:, 0:HF],
                        start=(kc == 0), stop=(kc == KC - 1),
                    )

            def exp_pair(c):
                # exp for j-chunks 2c, 2c+1 in one ACT instruction
                nc.scalar.activation(
                    e_sb[:, 2 * c:2 * c + 2, :], pdd[:, 2 * c:2 * c + 2, :],
                    EXP, bias=ebias[:],
                )

            def e8_copy(c):
                # batched fp8 half-copies of e for the wave-1 DoubleRow
                # denominator matmuls (Pool; off the DVE critical path)
                h = slice(0, 8) if c == 3 else slice(8, JC)
                nc.gpsimd.tensor_copy(e8_sb[:, h], e_sb[:, h])

            hfs = {}

            def mult_evict(jc):
                # eager psum->sbuf eviction for Pool-path chunks: frees the
                # ph bank as soon as possible (ACT, ahead of later exps)
                ph = phs.pop(jc)
                hf_sb = evp.tile([P, HF], f16, tag="evict")
                nc.scalar.activation(hf_sb[:], ph[:], COPY)
                hfs[jc] = hf_sb

            def mult(jc):
                # g_hi[jc] = e[jc] (bcast over F) * hfeat[jc], fp8 out
                e3 = _bcast_last(e_sb[:, jc, :], OUT_F)
                g3 = g_hi[:, jc, :].rearrange("p (h f) -> p h f", h=H)
                if jc in POOLJC:
                    hf_sb = hfs.pop(jc)
                    nc.gpsimd.tensor_tensor(
                        g3, hf_sb[:].rearrange("p (h f) -> p h f", h=H), e3,
                        op=MULT)
                else:
                    h3 = phs.pop(jc)[:].rearrange("p (h f) -> p h f", h=H)
                    nc.vector.tensor_tensor(g3, h3, e3, op=MULT)

            def mm_group(pF, ic, c):
                """Single-pass DoubleRow numerator and f16 denominator for
                j-chunk pair c (j-chunks 2c, 2c+1) into i-chunk ic."""
                ap = adjT_pair(c)
                nc.tensor.matmul(
                    pF[:], ap[:, :, ic * P:(ic + 1) * P],
                    g_hi[:, 2 * c:2 * c + 2, :],
                    start=(c == 0), stop=(c == JG - 1),
                    perf_mode=DR, skip_group_check=True,
                )
                for t in range(2):
                    jc = 2 * c + t
                    nc.tensor.matmul(
                        pden[:, ic, :], ap[:, t, ic * P:(ic + 1) * P],
                        e_sb[:, jc, :],
                        start=False, stop=(jc == JC - 1),
                        skip_group_check=True,
                    )

            def fin_mul(ic, pF, rc_ap, nsb, cols=None):
                sl = slice(0, HF) if cols is None else cols
                n3 = nsb[:, sl].rearrange("p (h f) -> p h f", f=OUT_F)
                nh = n3.shape[1]
                h0 = sl.start // OUT_F
                r3 = _bcast_last(rc_ap[:, h0:h0 + nh], OUT_F)
                if ic in POOLFIN:
                    nf_sb = evp.tile([P, HF], f16, tag="evict")
                    nc.scalar.activation(nf_sb[:, sl], pF[:, sl], COPY)
                    nc.gpsimd.tensor_tensor(
                        n3, nf_sb[:, sl].rearrange("p (h f) -> p h f",
                                                   f=OUT_F), r3,
                        op=MULT)
                else:
                    p3 = pF[:, sl].rearrange("p (h f) -> p h f", f=OUT_F)
                    nc.vector.tensor_tensor(n3, p3, r3, op=MULT)

            # --- wave 0: projection interleaved with ics 0..WAVE-1.
            # Aggregation runs LAG j-chunks behind projection; pair 7 is
            # deferred past the wave boundary so PE has useful work (ic2's
            # pairs 0-6) while the last multiplies drain. ---
            for jc in range(DSTLEAD):
                dst_mm(jc)
            exp_pair(0)
            for step in range(JC + LAG):
                if step < JC:
                    proj(step)
                    if DSTLEAD and step + DSTLEAD < JC:
                        dst_mm(step + DSTLEAD)
                    if step % 2 == 1 and (step + 1) // 2 < JC // 2:
                        # dsts lead by DSTLEAD, so pair (step+1)//2 is
                        # complete: issue its exp a full pair-step early
                        exp_pair((step + 1) // 2)
                    if step in POOLJC:
                        mult_evict(step)
                    mult(step)
                    if step in (7, JC - 1):
                        e8_copy(step // 2)
                ready = step - LAG
                if ready >= 0 and ready % 2 == 1:
                    c = ready // 2
                    if c < JG - 1:
                        for k in range(WAVE):
                            mm_group(pFs[k], k, c)

            NW1 = IC - WAVE
            pF1 = []
            rc1 = scr.tile([P, NW1, H], f32, tag="rc1")
            nsb1 = nsbp.tile([P, NW1, HF], f16, tag="nsb", name="nsb1")

            def num_dr(pF, ic, c, start=None):
                lhs2 = adjT_pair(c)[:, :, ic * P:(ic + 1) * P]
                nc.tensor.matmul(
                    pF[:], lhs2, g_hi[:, 2 * c:2 * c + 2, :],
                    start=(c == 0) if start is None else start,
                    stop=(c == JG - 1),
                    perf_mode=DR, skip_group_check=True)

            def den_dr(ic, c):
                nc.tensor.matmul(
                    pden[:, ic, :],
                    adjT_pair(c)[:, :, ic * P:(ic + 1) * P],
                    e8_sb[:, 2 * c:2 * c + 2, :], start=False,
                    stop=(c == JG - 1),
                    perf_mode=DR, skip_group_check=True)

            # --- wave boundary: mult-independent PE work (ic2/ic3 pairs
            # 0-6, wave-1 dens) covers the latency of the last exp->multiply
            # chains before pair 7 is consumed ---
            NBIC = 2 if BIC3 else 1
            for kk in range(NBIC):
                pF = psh.tile([P, HF], f32, tag="hfeat", name=f"pF1_{kk}")
                pF1.append(pF)
                for c in range(JG - 1):
                    num_dr(pF, WAVE + kk, c)
            for ic in range(WAVE, IC):
                for c in range(BDENS):
                    den_dr(ic, c)
            # wave0 pair 7, then recip + fins + store
            for k in range(WAVE):
                mm_group(pFs[k], k, JG - 1)
            for ic in range(WAVE, IC):
                for c in range(BDENS, JG):
                    den_dr(ic, c)
            rc0 = scr.tile([P, WAVE, H], f32, tag="rc0")
            nc.vector.reciprocal(rc0[:], pden[:, 0:WAVE, :])
            nsb0 = nsbp.tile([P, WAVE, HF], f16, tag="nsb", name="nsb0")
            for k in range(WAVE):
                fin_mul(k, pFs[k], rc0[:, k, :], nsb0[:, k])
            nc.sync.dma_start(out_d[:, 0:WAVE * HF], nsb0[:])
            nc.vector.reciprocal(rc1[:], pden[:, WAVE:IC, :])
            for kk in range(NBIC):
                num_dr(pF1[kk], WAVE + kk, JG - 1, start=False)

            def fin_store1(k):
                """Finalize wave-1 ic WAVE+k and store per schedule."""
                fin_mul(WAVE + k, pF1[k], rc1[:, k, :], nsb1[:, k])
                if k in (1, 3):
                    nc.sync.dma_start(
                        out_d[:, (WAVE + k - 1) * HF:(WAVE + k + 1) * HF],
                        nsb1[:, k - 1:k + 1])
                elif k == 4:
                    nc.sync.dma_start(
                        out_d[:, (WAVE + k) * HF:(WAVE + k + 1) * HF],
                        nsb1[:, k])

            for k in range(NBIC, NW1 - 1):
                ic = WAVE + k
                if k < 2:
                    pF = psh.tile([P, HF], f32, tag="hfeat", name=f"pF1_{k}")
                else:
                    pF = psnum.tile([P, HF], f32, tag="num", name=f"pF1_{k}")
                pF1.append(pF)
                for c in range(JG):
                    num_dr(pF, ic, c)
                    if c == 3 and k - NBIC + 1 >= 1:
                        fin_store1(k - NBIC)
            # last ic: optionally COLUMN-split into separate psum tiles;
            # fin of the first half overlaps the second half's matmuls
            k = NW1 - 1
            if FIN7SPLIT:
                pF7 = [psnum.tile([P, HF // 2], f32, tag="num", name="pF7a"),
                       psh.tile([P, HF // 2], f32, tag="hfeat", name="pF7b")]
                pF1.append(pF7[0])
                for hh in range(2):
                    sl = slice(hh * (HF // 2), (hh + 1) * (HF // 2))
                    for c in range(JG):
                        lhs2 = adjT_pair(c)[:, :, (IC - 1) * P:IC * P]
                        nc.tensor.matmul(
                            pF7[hh][:], lhs2,
                            g_hi[:, 2 * c:2 * c + 2, sl],
                            start=(c == 0), stop=(c == JG - 1),
                            perf_mode=DR, skip_group_check=True)
                        if hh == 0 and c == 3:
                            fin_store1(k - 1)
                        elif hh == 0 and c == 5 and NBIC == 2:
                            fin_store1(k - NBIC)
                    n3 = nsb1[:, k, sl].rearrange("p (h f) -> p h f",
                                                  f=OUT_F)
                    r3 = _bcast_last(rc1[:, k, 4 * hh:4 * hh + 4], OUT_F)
                    p3 = pF7[hh][:].rearrange("p (h f) -> p h f", f=OUT_F)
                    nc.vector.tensor_tensor(n3, p3, r3, op=MULT)
            else:
                pF = psnum.tile([P, HF], f32, tag="num", name="pF7")
                pF1.append(pF)
                for c in range(JG):
                    num_dr(pF, IC - 1, c)
                    if c == 3:
                        fin_store1(k - 1)
                    elif c == 5 and NBIC == 2:
                        fin_store1(k - NBIC)
                fin_mul(IC - 1, pF, rc1[:, k, :], nsb1[:, k])
            nc.sync.dma_start(
                out_d[:, (IC - 1) * HF:IC * HF], nsb1[:, k])

    nc.compile()
    return nc


def _get_nc():
    if "nc" not in _CACHE:
        _CACHE["nc"] = _build()
    return _CACHE["nc"]


def _make_in_maps(x, adj, weight, attn_dst):
    x = np.ascontiguousarray(np.asarray(x), dtype=np.float32)
    adj = np.asarray(adj)
    weight = np.ascontiguousarray(np.asarray(weight), dtype=np.float32)
    attn_dst = np.ascontiguousarray(np.asarray(attn_dst), dtype=np.float32)

    # fold attn_dst into the weight: wdst[k, h] = sum_f W[k, h*64+f]*adst[h, f]
    wdst = (weight.reshape(IN_F, H, OUT_F) * attn_dst[None]).sum(-1)

    # pack [W | wdst] -> [P, KC, HF+H]
    wfull = np.concatenate([weight, wdst], axis=1)        # [IN_F, WD]
    w_kp = np.ascontiguousarray(
        wfull.reshape(KC, P, WD).transpose(1, 0, 2).reshape(P, KC * WD)
    ).astype(F16)

    in_maps = []
    for core in range(NCORES):
        b = core // 2
        half = core % 2
        # xt layout [p, jc, kc, j']: x[b][jc*128 + j', kc*128 + p]
        xt = x[b].T.reshape(KC, P, JC, P)              # [kc, p, jc, j']
        xt_kp = np.ascontiguousarray(
            xt.transpose(1, 2, 0, 3).reshape(P, JC * KC * P)
        ).astype(F16)
        adjt = adj[b].T[:, half * ROWS:(half + 1) * ROWS]  # [N, ROWS]
        in_maps.append({
            "adjt": np.ascontiguousarray(adjt, dtype=np.float32).astype(FP8),
            "w": w_kp,
            "xt": xt_kp,
        })
    return in_maps


def _run_device(in_maps):
    from concourse import bass_utils

    nc = _get_nc()
    res = bass_utils.run_bass_kernel_spmd(
        nc, in_maps, core_ids=list(range(NCORES)))
    return [dict(r) for r in res.results]


def _run_device_subprocess(in_maps):
    """Fresh-process fallback: a wedged accelerator surfaces as
    NRT_EXEC_UNIT_UNRECOVERABLE and poisons the in-process PJRT client;
    a new process gets a fresh axon session and a reset device."""
    import os
    import pickle
    import subprocess
    import sys
    import tempfile

    d = tempfile.mkdtemp(prefix="gat_kernel_")
    inp = os.path.join(d, "in.pkl")
    outp = os.path.join(d, "out.pkl")
    with open(inp, "wb") as f:
        pickle.dump(in_maps, f)
    code = (
        "import pickle, sys\n"
        f"sys.path.insert(0, {os.path.dirname(os.path.abspath(__file__))!r})\n"
        "import kernel\n"
        f"in_maps = pickle.load(open({inp!r}, 'rb'))\n"
        f"pickle.dump(kernel._run_device(in_maps), open({outp!r}, 'wb'))\n"
    )
    env = dict(os.environ, GAT_KERNEL_SUBPROC="1")
    subprocess.run([sys.executable, "-c", code], check=True, env=env,
                   timeout=1200)
    with open(outp, "rb") as f:
        return pickle.load(f)


def kernel(x, adj, weight, attn_src, attn_dst):
    import os
    import time

    in_maps = _make_in_maps(x, adj, weight, attn_dst)
    try:
        results = _run_device(in_maps)
    except Exception:
        if os.environ.get("GAT_KERNEL_SUBPROC") == "1":
            raise
        time.sleep(2)
        results = _run_device_subprocess(in_maps)

    out = np.empty((B, N, HF), dtype=np.float32)
    for core in range(NCORES):
        b = core // 2
        half = core % 2
        res = results[core]["out"].astype(np.float32)      # [P, IC*HF]
        for q in range(IC):
            r0 = half * ROWS + q * P
            out[b, r0:r0 + P, :] = res[:, q * HF:(q + 1) * HF]
    return out


# revision 46
# speedup vs baseline: 1.0073x; 1.0057x over previous
"""GAT layer kernel for Trainium2, 8 NeuronCores.

Problem: nn_GATLayer (B=4, N=2048, IN_F=256, OUT_F=64, H=8).

Key algebra: softmax over j of (src[b,i,h] + dst[b,j,h]) masked by adj[b,i,j].
src[b,i,h] is constant over j, so it cancels in the softmax:
    out[b,i,(h,f)] = (adj[b,i,:] @ g[b,:, (h,f)]) / (adj[b,i,:] @ e[b,:,h])
with hfeat = x@W (per-head features), dst[j,h] = x[j,:] @ (W . attn_dst)[:,h],
e = exp(dst), g = e * hfeat.  attn_src is mathematically irrelevant.

Sharding: 8 cores = 4 batches x 2 row-halves of i (softmax is over j only,
so row-sharding of i needs no communication).

Numerics/perf design (final):
  - fp16 projection (x, W, wdst in fp16; psum f32).  wdst = W . attn_dst
    folded on host and packed into the same dram tensor as W (one DMA).
  - e = exp(dst + ln(3/16)) in fp16; the global 3/16 scale cancels in the
    softmax ratio and keeps g inside e4m3 range.
  - g = e*hfeat written DIRECTLY as fp8 e4m3 by the multiply; the numerator
    is a SINGLE fp8 DoubleRow pass (no hi/lo split, no cast/subtract
    pipeline).  Measured absmax-rel err ~1.5e-2 < 2e-2 gate.
  - Wave-0 denominators are plain f16 matmuls inside the aggregation loop;
    wave-1 denominators are DoubleRow matmuls on an fp8 copy of e and are
    hoisted (with one batched reciprocal) off the finalize tail chains.
  - dst matmuls are issued DSTLEAD j-chunks ahead of the hfeat matmuls so
    the exp's PSUM read never blocks the next chunk's PSUM write (the dep
    tracker is tile-granular).
  - Elementwise per j-chunk: exp batched per pair (ACT); the e*hfeat
    multiply runs on DVE straight from PSUM for most chunks, and via
    ACT-evict(psum->fp16 sbuf) + Pool multiply for POOLJC chunks.
    Finalize (num * 1/den) likewise DVE or ACT+Pool per POOLFIN.
  - Two waves of i-chunks: 2 interleaved with the projection (aggregation
    LAG j-chunks behind), 6 after, completions staggered so fins/stores
    drain progressively; the last i-chunk is column-split into two psum
    tiles so its first fin overlaps its second half's matmuls.
  - 8 input DMAs total (w, 3 xt slabs, 4 adjT groups) keep the shared
    HWDGE/DMA engines ahead of consumption; PE pre-warm dummy matmuls pin
    pe_busy_start early so the p-state ramp completes behind the input
    DMA latency.
"""

import numpy as np
import ml_dtypes

B, N, IN_F, OUT_F, H = 4, 2048, 256, 64, 8
HF = H * OUT_F            # 512 concat features
NCORES = 8
ROWS = B * N // NCORES    # 1024 destination rows per core
P = 128
IC = ROWS // P            # 8 i-chunks per core
JC = N // P               # 16 j-chunks
KC = IN_F // P            # 2 k-chunks
JG = 8                    # DoubleRow j-chunk pairs
AG = 4                    # adjT DMA groups (4 j-chunks each)
WAVE = 2                  # i-chunks in the first (interleaved) wave
NDUMMY = 2                # PE p-state pre-warm matmuls (sets pe_busy_start)
LAG = 9                   # j-chunks the aggregation trails projection by
ESCALE = 0.1875           # global scale on e; cancels in the softmax ratio
WD = HF + H               # packed W columns (hfeat + dst)

POOLJC = (2, 4, 6, 9, 11)       # j-chunks whose multiply goes ACT-evict+Pool
DSTLEAD = 2               # j-chunks the dst matmuls lead the hfeat matmuls by
BDENS = 4                 # wave-1 den pairs issued before wave0 pair 7
BIC3 = False              # ic3's pairs 0-6 run at the wave boundary
POOLFIN = (0, 2)                # i-chunks whose finalize goes ACT-evict+Pool
# xt DMA slabs (j-chunk ranges) interleaved with adjT groups:
#   w, s0, s1, g0, s2, g1, g2, g3
XSLABS = ((0, 2), (2, 8), (8, 16))
FIN7SPLIT = True          # column-split the last ic across two psum tiles

F16 = np.float16
FP8 = ml_dtypes.float8_e4m3

_CACHE = {}


def _bcast_last(ap, n):
    """View ap with an extra innermost broadcast (stride-0) dim of size n."""
    ap2 = ap.unsqueeze(len(ap.shape))
    return ap2.broadcast_to(tuple(ap.shape) + (n,))


def _build():
    import concourse.mybir as mybir
    import concourse.tile as tile
    from concourse import bacc

    f32 = mybir.dt.float32
    f16 = mybir.dt.float16
    fp8 = mybir.dt.float8e4
    MULT = mybir.AluOpType.mult
    DR = mybir.MatmulPerfMode.DoubleRow
    COPY = mybir.ActivationFunctionType.Copy
    EXP = mybir.ActivationFunctionType.Exp

    nc = bacc.Bacc(trn_type="TRN2", debug=False, target_bir_lowering=False)

    adjt_d = nc.dram_tensor("adjt", [N, ROWS], fp8, kind="ExternalInput")
    w_d = nc.dram_tensor("w", [P, KC * WD], f16, kind="ExternalInput")
    xt_d = nc.dram_tensor("xt", [P, JC * KC * P], f16, kind="ExternalInput")
    out_d = nc.dram_tensor("out", [P, IC * HF], f16, kind="ExternalOutput")

    with tile.TileContext(nc) as tc:
        with (
            tc.tile_pool(name="setup", bufs=1) as setup,
            tc.tile_pool(name="gpool", bufs=1) as gpool,
            tc.tile_pool(name="scratch", bufs=2) as scr,
            tc.tile_pool(name="evict", bufs=6) as evp,
            tc.tile_pool(name="adjT", bufs=1) as adjTp,
            tc.tile_pool(name="warm", bufs=1) as warmp,
            tc.tile_pool(name="ps_num", bufs=WAVE, space="PSUM") as psnum,
            tc.tile_pool(name="ps_h", bufs=5, space="PSUM") as psh,
            tc.tile_pool(name="ps_dd", bufs=1, space="PSUM") as psdd,
            tc.tile_pool(name="nsbp", bufs=2) as nsbp,
        ):
            # --- PE pre-warm: dummies into pF0; its first real matmul
            # re-starts the accumulation group, so no extra bank is used ---
            pFs = [psnum.tile([P, HF], f32, tag="num", name=f"pF0_{k}")
                   for k in range(WAVE)]
            junk = warmp.tile([P, 1], f16)
            nc.vector.memset(junk[:], 0.0)
            ebias = warmp.tile([P, 1], f32)
            nc.vector.memset(ebias[:], float(np.log(ESCALE)))
            junk_l = junk[:].broadcast_to((P, P))
            junk_rhs = junk[:].unsqueeze(1).broadcast_to((P, 4, P))
            for _ in range(NDUMMY):
                nc.tensor.matmul(pFs[0][:], junk_l, junk_rhs,
                                 start=True, stop=True, skip_group_check=True)

            # --- input streams via HWDGE (sync), EDF-ordered ---
            w_sb = setup.tile([P, KC, WD], f16)
            w_kc = [w_sb[:, kc] for kc in range(KC)]
            xT_sb = setup.tile([P, JC, KC, P], f16)
            adjT_g = []
            for G in range(AG):
                adjT_g.append(adjTp.tile([P, 4, ROWS], fp8, tag=f"adjt{G}",
                                         name=f"adjt{G}"))

            w_v = w_d.rearrange("p (kc n) -> p kc n", kc=KC)
            xt_v = xt_d.rearrange("p (jc kc j) -> p jc kc j", jc=JC, kc=KC)

            def load_slab(s):
                j0, j1 = XSLABS[s]
                nc.sync.dma_start(
                    xT_sb[:, j0:j1], xt_v[:, j0:j1])

            def load_adjt(G):
                nc.sync.dma_start(
                    adjT_g[G][:],
                    adjt_d[G * 4 * P:(G + 1) * 4 * P, :].rearrange(
                        "(jc jp) i -> jp jc i", jp=P),
                )

            def adjT_pair(c):
                # [P, 2, ROWS] view of j-chunk pair c
                t = 2 * (c % 2)
                return adjT_g[c // 2][:, t:t + 2]

            nc.sync.dma_start(w_sb[:], w_v[:])
            load_slab(0)
            load_slab(1)
            load_adjt(0)
            load_slab(2)
            for G in range(1, AG):
                load_adjt(G)

            def xT(kc, jc):
                return xT_sb[:, jc, kc, :]

            g_hi = gpool.tile([P, JC, HF], fp8)
            e_sb = gpool.tile([P, JC, H], f16)
            e8_sb = gpool.tile([P, JC, H], fp8)
            # dst logits AND denominators share ONE psum bank: dst's single
            # start=True marks the whole bank pending-zero, so den matmuls
            # accumulate with start=False onto zeroed bytes
            pdd = psdd.tile([P, JC + IC, H], f32)
            pden = pdd[:, JC:JC + IC, :]

            def pdst(jc):
                return pdd[:, jc, :]

            phs = {}

            def dst_mm(jc):
                for kc in range(KC):
                    nc.tensor.matmul(
                        pdst(jc), xT(kc, jc), w_kc[kc][:, HF:WD],
                        start=(jc == 0 and kc == 0), stop=(kc == KC - 1),
                        skip_group_check=True,
                    )

            def proj(jc):
                if DSTLEAD == 0:
                    dst_mm(jc)
                ph = psh.tile([P, HF], f32, tag="hfeat")
                phs[jc] = ph
                for kc in range(KC):
                    nc.tensor.matmul(
                        ph[:], xT(kc, jc), w_kc[kc][:, 0:HF],
                        start=(kc == 0), stop=(kc == KC - 1),
                    )

            def exp_pair(c):
                # exp for j-chunks 2c, 2c+1 in one ACT instruction
                nc.scalar.activation(
                    e_sb[:, 2 * c:2 * c + 2, :], pdd[:, 2 * c:2 * c + 2, :],
                    EXP, bias=ebias[:],
                )

            def e8_copy(c):
                # batched fp8 half-copies of e for the wave-1 DoubleRow
                # denominator matmuls (Pool; off the DVE critical path)
                h = slice(0, 8) if c == 3 else slice(8, JC)
                nc.gpsimd.tensor_copy(e8_sb[:, h], e_sb[:, h])

            hfs = {}

            def mult_evict(jc):
                # eager psum->sbuf eviction for Pool-path chunks: frees the
                # ph bank as soon as possible (ACT, ahead of later exps)
                ph = phs.pop(jc)
                hf_sb = evp.tile([P, HF], f16, tag="evict")
                nc.scalar.activation(hf_sb[:], ph[:], COPY)
                hfs[jc] = hf_sb

            def mult(jc):
                # g_hi[jc] = e[jc] (bcast over F) * hfeat[jc], fp8 out
                e3 = _bcast_last(e_sb[:, jc, :], OUT_F)
                g3 = g_hi[:, jc, :].rearrange("p (h f) -> p h f", h=H)
                if jc in POOLJC:
                    hf_sb = hfs.pop(jc)
                    nc.gpsimd.tensor_tensor(
                        g3, hf_sb[:].rearrange("p (h f) -> p h f", h=H), e3,
                        op=MULT)
                else:
                    h3 = phs.pop(jc)[:].rearrange("p (h f) -> p h f", h=H)
                    nc.vector.tensor_tensor(g3, h3, e3, op=MULT)

            def mm_group(pF, ic, c):
                """Single-pass DoubleRow numerator and f16 denominator for
                j-chunk pair c (j-chunks 2c, 2c+1) into i-chunk ic."""
                ap = adjT_pair(c)
                nc.tensor.matmul(
                    pF[:], ap[:, :, ic * P:(ic + 1) * P],
                    g_hi[:, 2 * c:2 * c + 2, :],
                    start=(c == 0), stop=(c == JG - 1),
                    perf_mode=DR, skip_group_check=True,
                )
                for t in range(2):
                    jc = 2 * c + t
                    nc.tensor.matmul(
                        pden[:, ic, :], ap[:, t, ic * P:(ic + 1) * P],
                        e_sb[:, jc, :],
                        start=False, stop=(jc == JC - 1),
                        skip_group_check=True,
                    )

            def fin_mul(ic, pF, rc_ap, nsb, cols=None):
                sl = slice(0, HF) if cols is None else cols
                n3 = nsb[:, sl].rearrange("p (h f) -> p h f", f=OUT_F)
                nh = n3.shape[1]
                h0 = sl.start // OUT_F
                r3 = _bcast_last(rc_ap[:, h0:h0 + nh], OUT_F)
                if ic in POOLFIN:
                    nf_sb = evp.tile([P, HF], f16, tag="evict")
                    nc.scalar.activation(nf_sb[:, sl], pF[:, sl], COPY)
                    nc.gpsimd.tensor_tensor(
                        n3, nf_sb[:, sl].rearrange("p (h f) -> p h f",
                                                   f=OUT_F), r3,
                        op=MULT)
                else:
                    p3 = pF[:, sl].rearrange("p (h f) -> p h f", f=OUT_F)
                    nc.vector.tensor_tensor(n3, p3, r3, op=MULT)

            # --- wave 0: projection interleaved with ics 0..WAVE-1.
            # Aggregation runs LAG j-chunks behind projection; pair 7 is
            # deferred past the wave boundary so PE has useful work (ic2's
            # pairs 0-6) while the last multiplies drain. ---
            for jc in range(DSTLEAD):
                dst_mm(jc)
            exp_pair(0)
            for step in range(JC + LAG):
                if step < JC:
                    proj(step)
                    if DSTLEAD and step + DSTLEAD < JC:
                        dst_mm(step + DSTLEAD)
                    if step % 2 == 1 and (step + 1) // 2 < JC // 2:
                        # dsts lead by DSTLEAD, so pair (step+1)//2 is
                        # complete: issue its exp a full pair-step early
                        exp_pair((step + 1) // 2)
                    if step in POOLJC:
                        mult_evict(step)
                    mult(step)
                    if step in (7, JC - 1):
                        e8_copy(step // 2)
                ready = step - LAG
                if ready >= 0 and ready % 2 == 1:
                    c = ready // 2
                    if c < JG - 1:
                        for k in range(WAVE):
                            mm_group(pFs[k], k, c)

            NW1 = IC - WAVE
            pF1 = []
            rc1 = scr.tile([P, NW1, H], f32, tag="rc1")
            nsb1 = nsbp.tile([P, NW1, HF], f16, tag="nsb", name="nsb1")

            def num_dr(pF, ic, c, start=None):
                lhs2 = adjT_pair(c)[:, :, ic * P:(ic + 1) * P]
                nc.tensor.matmul(
                    pF[:], lhs2, g_hi[:, 2 * c:2 * c + 2, :],
                    start=(c == 0) if start is None else start,
                    stop=(c == JG - 1),
                    perf_mode=DR, skip_group_check=True)

            def den_dr(ic, c):
                nc.tensor.matmul(
                    pden[:, ic, :],
                    adjT_pair(c)[:, :, ic * P:(ic + 1) * P],
                    e8_sb[:, 2 * c:2 * c + 2, :], start=False,
                    stop=(c == JG - 1),
                    perf_mode=DR, skip_group_check=True)

            # --- wave boundary: mult-independent PE work (ic2/ic3 pairs
            # 0-6, wave-1 dens) covers the latency of the last exp->multiply
            # chains before pair 7 is consumed ---
            NBIC = 2 if BIC3 else 1
            for kk in range(NBIC):
                pF = psh.tile([P, HF], f32, tag="hfeat", name=f"pF1_{kk}")
                pF1.append(pF)
                for c in range(JG - 1):
                    num_dr(pF, WAVE + kk, c)
            for ic in range(WAVE, IC):
                for c in range(BDENS):
                    den_dr(ic, c)
            # wave0 pair 7, then recip + fins + store
            for k in range(WAVE):
                mm_group(pFs[k], k, JG - 1)
            for ic in range(WAVE, IC):
                for c in range(BDENS, JG):
                    den_dr(ic, c)
            rc0 = scr.tile([P, WAVE, H], f32, tag="rc0")
            nc.vector.reciprocal(rc0[:], pden[:, 0:WAVE, :])
            nsb0 = nsbp.tile([P, WAVE, HF], f16, tag="nsb", name="nsb0")
            for k in range(WAVE):
                fin_mul(k, pFs[k], rc0[:, k, :], nsb0[:, k])
            nc.sync.dma_start(out_d[:, 0:WAVE * HF], nsb0[:])
            nc.vector.reciprocal(rc1[:], pden[:, WAVE:IC, :])
            for kk in range(NBIC):
                num_dr(pF1[kk], WAVE + kk, JG - 1, start=False)

            def fin_store1(k):
                """Finalize wave-1 ic WAVE+k and store per schedule."""
                fin_mul(WAVE + k, pF1[k], rc1[:, k, :], nsb1[:, k])
                if k in (1, 3):
                    nc.sync.dma_start(
                        out_d[:, (WAVE + k - 1) * HF:(WAVE + k + 1) * HF],
                        nsb1[:, k - 1:k + 1])
                elif k == 4:
                    nc.sync.dma_start(
                        out_d[:, (WAVE + k) * HF:(WAVE + k + 1) * HF],
                        nsb1[:, k])

            for k in range(NBIC, NW1 - 1):
                ic = WAVE + k
                if k < 2:
                    pF = psh.tile([P, HF], f32, tag="hfeat", name=f"pF1_{k}")
                else:
                    pF = psnum.tile([P, HF], f32, tag="num", name=f"pF1_{k}")
                pF1.append(pF)
                for c in range(JG):
                    num_dr(pF, ic, c)
                    if c == 3 and k - NBIC + 1 >= 1:
                        fin_store1(k - NBIC)
            # last ic: optionally COLUMN-split into separate psum tiles;
            # fin of the first half overlaps the second half's matmuls
            k = NW1 - 1
            if FIN7SPLIT:
                pF7 = [psnum.tile([P, HF // 2], f32, tag="num", name="pF7a"),
                       psh.tile([P, HF // 2], f32, tag="hfeat", name="pF7b")]
                pF1.append(pF7[0])
                for hh in range(2):
                    sl = slice(hh * (HF // 2), (hh + 1) * (HF // 2))
                    for c in range(JG):
                        lhs2 = adjT_pair(c)[:, :, (IC - 1) * P:IC * P]
                        nc.tensor.matmul(
                            pF7[hh][:], lhs2,
                            g_hi[:, 2 * c:2 * c + 2, sl],
                            start=(c == 0), stop=(c == JG - 1),
                            perf_mode=DR, skip_group_check=True)
                        if hh == 0 and c == 3:
                            fin_store1(k - 1)
                        elif hh == 0 and c == 5 and NBIC == 2:
                            fin_store1(k - NBIC)
                    n3 = nsb1[:, k, sl].rearrange("p (h f) -> p h f",
                                                  f=OUT_F)
                    r3 = _bcast_last(rc1[:, k, 4 * hh:4 * hh + 4], OUT_F)
                    p3 = pF7[hh][:].rearrange("p (h f) -> p h f", f=OUT_F)
                    nc.vector.tensor_tensor(n3, p3, r3, op=MULT)
            else:
                pF = psnum.tile([P, HF], f32, tag="num", name="pF7")
                pF1.append(pF)
                for c in range(JG):
                    num_dr(pF, IC - 1, c)
                    if c == 3:
                        fin_store1(k - 1)
                    elif c == 5 and NBIC == 2:
                        fin_store1(k - NBIC)
                fin_mul(IC - 1, pF, rc1[:, k, :], nsb1[:, k])
            nc.sync.dma_start(
                out_d[:, (IC - 1) * HF:IC * HF], nsb1[:, k])

    nc.compile()
    return nc


def _get_nc():
    if "nc" not in _CACHE:
        _CACHE["nc"] = _build()
    return _CACHE["nc"]


def _make_in_maps(x, adj, weight, attn_dst):
    x = np.ascontiguousarray(np.asarray(x), dtype=np.float32)
    adj = np.asarray(adj)
    weight = np.ascontiguousarray(np.asarray(weight), dtype=np.float32)
    attn_dst = np.ascontiguousarray(np.asarray(attn_dst), dtype=np.float32)

    # fold attn_dst into the weight: wdst[k, h] = sum_f W[k, h*64+f]*adst[h, f]
    wdst = (weight.reshape(IN_F, H, OUT_F) * attn_dst[None]).sum(-1)

    # pack [W | wdst] -> [P, KC, HF+H]
    wfull = np.concatenate([weight, wdst], axis=1)        # [IN_F, WD]
    w_kp = np.ascontiguousarray(
        wfull.reshape(KC, P, WD).transpose(1, 0, 2).reshape(P, KC * WD)
    ).astype(F16)

    in_maps = []
    for core in range(NCORES):
        b = core // 2
        half = core % 2
        # xt layout [p, jc, kc, j']: x[b][jc*128 + j', kc*128 + p]
        xt = x[b].T.reshape(KC, P, JC, P)              # [kc, p, jc, j']
        xt_kp = np.ascontiguousarray(
            xt.transpose(1, 2, 0, 3).reshape(P, JC * KC * P)
        ).astype(F16)
        adjt = adj[b].T[:, half * ROWS:(half + 1) * ROWS]  # [N, ROWS]
        in_maps.append({
            "adjt": np.ascontiguousarray(adjt, dtype=np.float32).astype(FP8),
            "w": w_kp,
            "xt": xt_kp,
        })
    return in_maps


def _run_device(in_maps):
    from concourse import bass_utils

    nc = _get_nc()
    res = bass_utils.run_bass_kernel_spmd(
        nc, in_maps, core_ids=list(range(NCORES)))
    return [dict(r) for r in res.results]


def _run_device_subprocess(in_maps):
    """Fresh-process fallback: a wedged accelerator surfaces as
    NRT_EXEC_UNIT_UNRECOVERABLE and poisons the in-process PJRT client;
    a new process gets a fresh axon session and a reset device."""
    import os
    import pickle
    import subprocess
    import sys
    import tempfile

    d = tempfile.mkdtemp(prefix="gat_kernel_")
    inp = os.path.join(d, "in.pkl")
    outp = os.path.join(d, "out.pkl")
    with open(inp, "wb") as f:
        pickle.dump(in_maps, f)
    code = (
        "import pickle, sys\n"
        f"sys.path.insert(0, {os.path.dirname(os.path.abspath(__file__))!r})\n"
        "import kernel\n"
        f"in_maps = pickle.load(open({inp!r}, 'rb'))\n"
        f"pickle.dump(kernel._run_device(in_maps), open({outp!r}, 'wb'))\n"
    )
    env = dict(os.environ, GAT_KERNEL_SUBPROC="1")
    subprocess.run([sys.executable, "-c", code], check=True, env=env,
                   timeout=1200)
    with open(outp, "rb") as f:
        return pickle.load(f)


def kernel(x, adj, weight, attn_src, attn_dst):
    import os
    import time

    in_maps = _make_in_maps(x, adj, weight, attn_dst)
    try:
        results = _run_device(in_maps)
    except Exception:
        if os.environ.get("GAT_KERNEL_SUBPROC") == "1":
            raise
        time.sleep(2)
        results = _run_device_subprocess(in_maps)

    out = np.empty((B, N, HF), dtype=np.float32)
    for core in range(NCORES):
        b = core // 2
        half = core % 2
        res = results[core]["out"].astype(np.float32)      # [P, IC*HF]
        for q in range(IC):
            r0 = half * ROWS + q * P
            out[b, r0:r0 + P, :] = res[:, q * HF:(q + 1) * HF]
    return out
